# revision 25
# baseline (speedup 1.0000x reference)
"""Trainium2 Bass kernel for nn_BispectrumPool.

Math (validated vs reference):
  F = FFT_8 along the group axis. beta[k] = F1*F[k]*conj(F[1+k mod 8]).
  Due to conjugate symmetry of the real-input FFT:
    beta4=beta3, beta5=beta2, beta6=beta1, beta7=beta0 (real), Im(beta0)=0
  -> only 7 distinct nonzero features per channel:
     [beta0r, beta1r, beta1i, beta2r, beta2i, beta3r, beta3i]
  with (b1,b2)=(Re,Im)F1, (b3,b4)=F2, (b5,b6)=F3, b7=F4(real), b0=F0:
     beta0r = b0*(b1^2+b2^2)
     beta1  = F1^2*conj(F2):    Gr=b1^2-b2^2, Gi=2b1b2
     beta2  = F2*H, H=F1*conj(F3): Hr=b1b5+b2b6, Hi=b2b5-b1b6
     beta3  = F4*K, K=F1*F3:       Kr=b1b5-b2b6, Ki=b2b5+b1b6

Square trick: instead of forming U,V layouts and multiplying, form ONE
W layout  [b2, b1, b1+b2, b1+b5, b5, b6, b2+b6, 0]  and square it on DVE.
All quadratics except Hi,Ki are linear combos of those squares; Hi,Ki come
from one extra 32-partition product [b2b5, b1b6]. This kills one form
matmul and one PSUM evacuation per block vs the direct approach.

Per (b, 448-col chunk, 16-channel block q):
  PE : W, A into one 2-bank PSUM tile; Ca (from squares); Cb (from hk);
       R1/R2 combine; conv accumulate (bias via ones-row, fp16 weights)
  ACT: ONE strided copy evacuates W+A -> fp16 SBUF; ln; relu on odd q
  DVE: sq and hk products (fp16 SBUF), t1, t2 (PSUM operand), relu on
       even q.  (sq on GPSIMD measured slower: Pool shares DVE's SBUF
       port and the contention cost more than DVE's extra op.)
  SP : x-in DMA, yt-out DMA

Distribution: pure data parallel, batch 16 -> 2 per core on 8 cores.
"""

import numpy as np

C, G = 64, 8
HWP = 56 * 56            # 3136
S = 448                  # chunk width (3136 = 7*448)
NCHUNK = HWP // S        # 7
NCORES = 8
BPC = 2                  # batches per core
NQ = 4                   # channel blocks of 16


def _form_rows():
    g = np.arange(G)
    B1 = np.cos(2 * np.pi * g / G)
    B2 = -np.sin(2 * np.pi * g / G)
    B3 = np.cos(4 * np.pi * g / G)
    B4 = -np.sin(4 * np.pi * g / G)
    B5 = np.cos(6 * np.pi * g / G)
    B6 = -np.sin(6 * np.pi * g / G)
    B7 = np.cos(np.pi * g)
    B0 = np.ones(G)
    # W blocks: [b2, b1, b1+b2, b1+b5, b5, b6, b2+b6, 0]
    W = np.stack([B2, B1, B1 + B2, B1 + B5, B5, B6, B2 + B6, np.zeros(G)])
    # A blocks: [b0, b3, b4, b3, b4, b3, b7, b4]
    #   (blocks 4..6 = [b4, b3, b7] so t2 can slice a16 at partition 64)
    A = np.stack([B0, B3, B4, B3, B4, B3, B7, B4])
    return W, A


def _combine_mats():
    # sq blocks: [s0..s6] = squares of [b2, b1, b1+b2, b1+b5, b5, b6, b2+b6]
    # Ca out blocks (paired with A): [S+, Gr, Gi, Gi, Gr, Hr, Kr, Hr]
    #   S+ = s1+s0;  Gr = s1-s0;  Gi = s2-s1-s0
    #   Hr = (s3-s1-s4 + s6-s0-s5)/2;  Kr = (s3-s1-s4 - s6+s0+s5)/2
    Wc_a = np.zeros((8, 7))
    Wc_a[0, 0], Wc_a[0, 1] = 1, 1                      # S+
    Wc_a[1, 0], Wc_a[1, 1] = -1, 1                     # Gr
    Wc_a[2, 0], Wc_a[2, 1], Wc_a[2, 2] = -1, -1, 1     # Gi
    Wc_a[3, 0], Wc_a[3, 1], Wc_a[3, 2] = -1, -1, 1     # Gi
    Wc_a[4, 0], Wc_a[4, 1] = -1, 1                     # Gr
    Wc_a[5] = [-0.5, -0.5, 0, 0.5, -0.5, -0.5, 0.5]    # Hr
    Wc_a[6] = [0.5, -0.5, 0, 0.5, -0.5, 0.5, -0.5]     # Kr
    Wc_a[7] = [-0.5, -0.5, 0, 0.5, -0.5, -0.5, 0.5]    # Hr
    # hk blocks: [b2b5, b1b6]
    # Cb out blocks (paired with a16[64:112]=[b4,b3,b7]): [Hi, Hi, Ki]
    #   Hi = hk0-hk1;  Ki = hk0+hk1
    Wc_b = np.zeros((3, 2))
    Wc_b[0] = [1, -1]
    Wc_b[1] = [1, -1]
    Wc_b[2] = [1, 1]
    # t1 products: [b0S+, b3Gr, b4Gi, b3Gi, b4Gr, b3Hr, b7Kr, b4Hr]
    # t2 products: [b4Hi, b3Hi, b7Ki]
    # features: [beta0r, beta1r, beta1i, beta2r, beta2i, beta3r, beta3i]
    Wr_1 = np.zeros((7, 8))
    Wr_1[0, 0] = 1                            # beta0r = b0S+
    Wr_1[1, 1] = Wr_1[1, 2] = 1               # beta1r = b3Gr + b4Gi
    Wr_1[2, 3], Wr_1[2, 4] = 1, -1            # beta1i = b3Gi - b4Gr
    Wr_1[3, 5] = 1                            # beta2r = b3Hr - b4Hi
    Wr_1[4, 7] = 1                            # beta2i = b4Hr + b3Hi
    Wr_1[5, 6] = 1                            # beta3r = b7Kr
    Wr_2 = np.zeros((7, 3))
    Wr_2[3, 0] = -1                           # beta2r -= b4Hi
    Wr_2[4, 1] = 1                            # beta2i += b3Hi
    Wr_2[6, 2] = 1                            # beta3i = b7Ki
    return Wc_a, Wc_b, Wr_1, Wr_2


def _block_diag_lhsT(n_in_blocks, blk=16, coef=None):
    """lhsT[k_partition, m] for a block-structured map."""
    n_out = len(coef)
    lhsT = np.zeros((n_in_blocks * blk, n_out * blk), dtype=np.float32)
    for mb in range(n_out):
        for kb in range(n_in_blocks):
            if coef[mb][kb] != 0.0:
                for c in range(blk):
                    lhsT[kb * blk + c, mb * blk + c] = coef[mb][kb]
    return lhsT


def _build_consts():
    W, A = _form_rows()
    Wc_a, Wc_b, Wr_1, Wr_2 = _combine_mats()

    # form matmuls: input partitions = (16c x 8g), c-major.
    def form_lhsT(rows):
        n_out = rows.shape[0]
        lhsT = np.zeros((128, n_out * 16), dtype=np.float32)
        for j in range(n_out):
            for c in range(16):
                for g in range(G):
                    lhsT[c * G + g, j * 16 + c] = rows[j, g]
        return lhsT

    cW = form_lhsT(W)                                        # [128, 128]
    cA = form_lhsT(A)                                        # [128, 128]
    cCa = _block_diag_lhsT(7, coef=Wc_a)                     # [112, 128]
    cCb = _block_diag_lhsT(2, coef=Wc_b)                     # [32, 48]
    cR1 = _block_diag_lhsT(8, coef=Wr_1)                     # [128, 112]
    cR2 = _block_diag_lhsT(3, coef=Wr_2)                     # [48, 112]
    return cW, cA, cCa, cCb, cR1, cR2


def _fold_weights(conv_w, conv_b):
    w = conv_w.reshape(64, C, 16)
    W7 = np.zeros((64, C, 7), dtype=np.float64)
    W7[..., 0] = w[..., 0] + w[..., 7]
    W7[..., 1] = w[..., 1] + w[..., 6]
    W7[..., 2] = w[..., 9] + w[..., 14]
    W7[..., 3] = w[..., 2] + w[..., 5]
    W7[..., 4] = w[..., 10] + w[..., 13]
    W7[..., 5] = w[..., 3] + w[..., 4]
    W7[..., 6] = w[..., 11] + w[..., 12]
    # conv lhsT per q: [112 = (7f x 16c), 64], packed side by side
    wf = np.zeros((112, NQ * 64), dtype=np.float32)
    for q in range(NQ):
        for f in range(7):
            for cl in range(16):
                wf[f * 16 + cl, q * 64:(q + 1) * 64] = W7[:, q * 16 + cl, f]
    return wf


_PROG_CACHE = {}


def _build_program(loop_n=1):
    import concourse.bass as bass
    import concourse.bacc as bacc
    import concourse.tile as tile
    import concourse.mybir as mybir

    f32 = mybir.dt.float32
    f32r = mybir.dt.float32r
    f16 = mybir.dt.float16
    nc = bacc.Bacc("TRN2", target_bir_lowering=False, debug=False,
                   num_devices=NCORES)

    x_d = nc.dram_tensor("x", [BPC, C * G, HWP], f32r, kind="ExternalInput").ap()
    cW_d = nc.dram_tensor("cW", [128, 128], f32r, kind="ExternalInput").ap()
    cA_d = nc.dram_tensor("cA", [128, 128], f32r, kind="ExternalInput").ap()
    cCa_d = nc.dram_tensor("cCa", [112, 128], f16, kind="ExternalInput").ap()
    cCb_d = nc.dram_tensor("cCb", [32, 48], f16, kind="ExternalInput").ap()
    cR1_d = nc.dram_tensor("cR1", [128, 112], f16, kind="ExternalInput").ap()
    cR2_d = nc.dram_tensor("cR2", [48, 112], f16, kind="ExternalInput").ap()
    wf_d = nc.dram_tensor("wf", [112, NQ * 64], f16, kind="ExternalInput").ap()
    bias_d = nc.dram_tensor("bias", [64, 1], f32, kind="ExternalInput").ap()
    y_d = nc.dram_tensor("y", [BPC, 64, HWP], f32, kind="ExternalOutput").ap()

    LN = mybir.ActivationFunctionType.Ln
    RELU = mybir.ActivationFunctionType.Relu
    IDENT = mybir.ActivationFunctionType.Identity
    MAX = mybir.AluOpType.max
    ADD = mybir.AluOpType.add

    with tile.TileContext(nc) as tc:
        with (
            tc.tile_pool(name="consts", bufs=1) as cpool,
            tc.tile_pool(name="xin", bufs=4) as xpool,
            tc.tile_pool(name="sb", bufs=5) as sbpool,
            tc.tile_pool(name="rl", bufs=4) as rlpool,
            tc.tile_pool(name="yt", bufs=3) as ypool,
            tc.tile_pool(name="psWA", bufs=2, space="PSUM") as psWA,
            tc.tile_pool(name="psC", bufs=1, space="PSUM") as psC,
            tc.tile_pool(name="psR", bufs=1, space="PSUM") as psR,
            tc.tile_pool(name="psY", bufs=1, space="PSUM") as psY,
        ):
            # --- load constants once ---
            cW = cpool.tile([128, 128], f32r, tag="cW")
            cA = cpool.tile([128, 128], f32r, tag="cA")
            cCa = cpool.tile([112, 128], f16, tag="cCa")
            cCb = cpool.tile([32, 48], f16, tag="cCb")
            cR1 = cpool.tile([128, 112], f16, tag="cR1")
            cR2 = cpool.tile([48, 112], f16, tag="cR2")
            # second copy of cR2 at partition base 64 (matmul requires
            # lhsT and rhs to share a base partition; R2b's rhs sits at 64)
            cR2x = cpool.tile([112, 112], f16, tag="cR2x")
            nc.sync.dma_start(out=cR2x[64:112, :], in_=cR2_d)
            wf = cpool.tile([112, NQ * 64], f16, tag="wf")
            bias = cpool.tile([64, 1], f32, tag="bias")
            for t, d in [(cW, cW_d), (cA, cA_d), (cCa, cCa_d), (cCb, cCb_d),
                         (cR1, cR1_d), (cR2, cR2_d), (wf, wf_d),
                         (bias, bias_d)]:
                nc.sync.dma_start(out=t[:], in_=d)

            # Dummy Ln before the loop: makes act-table set 5 (natural_log,
            # which contains Ln+Copy+Identity) resident on the loop-entry
            # path, so the fixpoint hoists BOTH per-iteration
            # InstLoadActFuncSet (2.57us/iter on ACT) out of the loop.
            # The result is stored to y_d[0,0,0:1] (overwritten by the loop)
            # purely so the op isn't dead-code-eliminated.
            warm = cpool.tile([1, 1], f32, tag="warmup")
            nc.scalar.activation(warm[:], bias[0:1, 0:1], LN, bias=1.0)
            nc.sync.dma_start(out=y_d[0, 0:1, 0:1], in_=warm[:])

            import contextlib
            loop_cm = (tc.For_i(0, loop_n, 1) if loop_n > 1
                       else contextlib.nullcontext())
            with loop_cm:
              for b in range(BPC):
                  for jj in range(0, NCHUNK, 2):
                    w = 2 if jj + 1 < NCHUNK else 1
                    sj = jj * S
                    # batched x loads: one [128, w*448] DMA per channel
                    # block covers w spatial chunks (fewer, larger DMAs)
                    xts = []
                    for q in range(NQ):
                        xt = xpool.tile([128, w, S], f32r, tag=f"x{q}")
                        nc.sync.dma_start(
                            out=xt[:], in_=x_d[b, 128 * q:128 * (q + 1),
                                               sj:sj + w * S])
                        xts.append(xt)
                    yt2 = ypool.tile([64, w, S], f32, tag="yt")
                    for dj in range(w):
                      j = jj + dj
                      s0 = j * S
                      pY = None
                      for q in range(NQ):
                          xq = xts[q][:, dj]
                          # form matmuls into one 2-bank PSUM tile
                          pWA = psWA.tile([128, 2, 512], f32, tag="wa")
                          nc.tensor.matmul(pWA[:, 0, 0:S], cW[:], xq)
                          nc.tensor.matmul(pWA[:, 1, 0:S], cA[:], xq)
                          # single strided evacuation W+A -> fp16
                          wa16 = sbpool.tile([128, 2, S], f16, tag="wa16")
                          nc.scalar.copy(wa16[:], pWA[:, :, 0:S])
                          # squares and hk product on DVE (all-fp16-SBUF).
                          # hk reads its two operand blocks at different
                          # partition bases directly (engine reads follow
                          # the src mem-pattern) -- avoids the 2us-latency
                          # SBUF-to-SBUF partner-alignment DMA that stalled
                          # the per-block dependency chain.
                          sq16 = sbpool.tile([112, S], f16, tag="sq")
                          nc.vector.tensor_mul(sq16[:], wa16[0:112, 0],
                                               wa16[0:112, 0])
                          hkb = sbpool.tile([32, S], f16, tag="hkb")
                          nc.sync.dma_start(out=hkb[:], in_=wa16[64:96, 0])
                          hk16 = sbpool.tile([32, S], f16, tag="hk")
                          nc.vector.tensor_mul(hk16[:], wa16[0:32, 0],
                                               hkb[:])
                          # quadratic combines (fp16 matmuls).  Cb is
                          # column-packed [112, S/2] (col-halves stacked at
                          # partitions 0 and 64): DVE cost scales with
                          # free-dim cols only, so t2 at half width is ~2x
                          # cheaper than the [48, S] layout.
                          H = S // 2
                          pCa = psC.tile([128, S], f32, tag="ca")
                          pCb = psC.tile([112, H], f32, tag="cb")
                          nc.tensor.matmul(pCa[:], cCa[:], sq16[:])
                          nc.tensor.matmul(pCb[0:48, :], cCb[:],
                                           hk16[:, 0:H])
                          nc.tensor.matmul(pCb[64:112, :], cCb[:],
                                           hk16[:, H:S])
                          # t2 partner (A blocks 4-6) rearranged to the
                          # same packed layout via SBUF-SBUF DMA
                          ap2 = sbpool.tile([112, H], f16, tag="ap2")
                          nc.sync.dma_start(out=ap2[0:48, :],
                                            in_=wa16[64:112, 1, 0:H])
                          nc.sync.dma_start(out=ap2[64:112, :],
                                            in_=wa16[64:112, 1, H:S])
                          # round-2 products (one PSUM operand each)
                          t1 = sbpool.tile([128, S], f16, tag="t1")
                          t2 = sbpool.tile([112, H], f16, tag="t2")
                          nc.vector.tensor_mul(t1[:], pCa[:], wa16[:, 1])
                          nc.vector.tensor_mul(t2[:], pCb[:], ap2[:])
                          # beta combine per column-half so every PSUM
                          # region sees exactly one start and one stop
                          pR = psR.tile([112, S], f32, tag="rpre")
                          nc.tensor.matmul(pR[:, 0:H], cR1[:], t1[:, 0:H],
                                           start=True, stop=False)
                          nc.tensor.matmul(pR[:, 0:H], cR2[:], t2[0:48, :],
                                           start=False, stop=True)
                          nc.tensor.matmul(pR[:, H:S], cR1[:], t1[:, H:S],
                                           start=True, stop=False)
                          nc.tensor.matmul(pR[:, H:S], cR2x[64:112, :],
                                           t2[64:112, :],
                                           start=False, stop=True)
                          # ln(1+relu(x)) = max(0, ln(1+x)): MAX on this HW
                          # suppresses NaN (and -inf < 0), so Ln straight
                          # from PSUM (one ACT pass) + a cheap 4x-mode fp16
                          # max on DVE replaces the relu+ln two-pass chain.
                          ln16 = sbpool.tile([112, S], f16, tag="ln16")
                          nc.scalar.activation(ln16[:], pR[:], LN, bias=1.0)
                          rl = rlpool.tile([112, S], f16, tag="rl")
                          nc.vector.tensor_scalar_max(rl[:], ln16[:], 0.0)
                          # conv accumulate
                          if q == 0:
                              pY = psY.tile([64, S], f32, tag="y")
                          nc.tensor.matmul(pY[:],
                                           wf[:, q * 64:(q + 1) * 64],
                                           rl[:], start=(q == 0),
                                           stop=(q == NQ - 1))
                      # evacuate y with bias add on ACT (DVE is the
                      # bottleneck engine); store batched over w chunks
                      nc.scalar.activation(yt2[:, dj], pY[:], IDENT,
                                           bias=bias[:, 0:1])
                    nc.sync.dma_start(out=y_d[b, :, sj:sj + w * S],
                                      in_=yt2[:])
    nc.compile()
    return nc


def kernel(x, conv_w, conv_b):
    from concourse.bass_utils import run_bass_kernel_spmd

    x = np.asarray(x)
    conv_w = np.asarray(conv_w)
    conv_b = np.asarray(conv_b)
    B = x.shape[0]
    xr = np.ascontiguousarray(
        x.reshape(B, C * G, HWP).astype(np.float32))
    cWm, cA, cCa, cCb, cR1, cR2 = _build_consts()
    wf = _fold_weights(conv_w.astype(np.float64), conv_b.astype(np.float64))

    key = "prog"
    if key not in _PROG_CACHE:
        _PROG_CACHE[key] = _build_program()
    nc = _PROG_CACHE[key]

    f16 = np.float16
    consts = dict(cW=cWm, cA=cA,
                  cCa=np.ascontiguousarray(cCa.astype(f16)),
                  cCb=np.ascontiguousarray(cCb.astype(f16)),
                  cR1=np.ascontiguousarray(cR1.astype(f16)),
                  cR2=np.ascontiguousarray(cR2.astype(f16)),
                  wf=np.ascontiguousarray(wf.astype(f16)),
                  bias=np.ascontiguousarray(
                      conv_b.astype(np.float32).reshape(64, 1)))
    in_maps = []
    for i in range(NCORES):
        m = dict(consts)
        m["x"] = np.ascontiguousarray(xr[i * BPC:(i + 1) * BPC])
        in_maps.append(m)

    res = run_bass_kernel_spmd(nc, in_maps, core_ids=list(range(NCORES)))
    y = np.concatenate([res.results[i]["y"] for i in range(NCORES)], axis=0)
    return np.ascontiguousarray(y.reshape(B, 64, 56, 56).astype(np.float32))



# revision 28
# speedup vs baseline: 1.0847x; 1.0847x over previous
"""Trainium2 Bass kernel for nn_BispectrumPool.

Math (validated vs reference):
  F = FFT_8 along the group axis. beta[k] = F1*F[k]*conj(F[1+k mod 8]).
  Due to conjugate symmetry of the real-input FFT:
    beta4=beta3, beta5=beta2, beta6=beta1, beta7=beta0 (real), Im(beta0)=0
  -> only 7 distinct nonzero features per channel:
     [beta0r, beta1r, beta1i, beta2r, beta2i, beta3r, beta3i]
  with (b1,b2)=(Re,Im)F1, (b3,b4)=F2, (b5,b6)=F3, b7=F4(real), b0=F0:
     beta0r = b0*(b1^2+b2^2)
     beta1  = F1^2*conj(F2):    Gr=b1^2-b2^2, Gi=2b1b2
     beta2  = F2*H, H=F1*conj(F3): Hr=b1b5+b2b6, Hi=b2b5-b1b6
     beta3  = F4*K, K=F1*F3:       Kr=b1b5-b2b6, Ki=b2b5+b1b6

Square trick: instead of forming U,V layouts and multiplying, form ONE
W layout  [b2, b1, b1+b2, b1+b5, b5, b6, b2+b6, 0]  and square it on DVE.
All quadratics except Hi,Ki are linear combos of those squares; Hi,Ki come
from one extra 32-partition product [b2b5, b1b6]. This kills one form
matmul and one PSUM evacuation per block vs the direct approach.

Per (b, 448-col chunk, 16-channel block q):
  PE : W, A into one 2-bank PSUM tile; Ca (from squares); Cb (from hk);
       R1/R2 combine; conv accumulate (bias via ones-row, fp16 weights)
  ACT: ONE strided copy evacuates W+A -> fp16 SBUF; ln; relu on odd q
  DVE: sq and hk products (fp16 SBUF), t1, t2 (PSUM operand), relu on
       even q.  (sq on GPSIMD measured slower: Pool shares DVE's SBUF
       port and the contention cost more than DVE's extra op.)
  SP : x-in DMA, yt-out DMA

Distribution: pure data parallel, batch 16 -> 2 per core on 8 cores.
"""

import numpy as np

C, G = 64, 8
HWP = 56 * 56            # 3136
S = 448                  # chunk width (3136 = 7*448)
NCHUNK = HWP // S        # 7
NCORES = 8
BPC = 2                  # batches per core
NQ = 4                   # channel blocks of 16


def _form_rows():
    g = np.arange(G)
    B1 = np.cos(2 * np.pi * g / G)
    B2 = -np.sin(2 * np.pi * g / G)
    B3 = np.cos(4 * np.pi * g / G)
    B4 = -np.sin(4 * np.pi * g / G)
    B5 = np.cos(6 * np.pi * g / G)
    B6 = -np.sin(6 * np.pi * g / G)
    B7 = np.cos(np.pi * g)
    B0 = np.ones(G)
    # W blocks: [b2, b1, b1+b2, b1+b5, b5, b6, b2+b6, 0]
    W = np.stack([B2, B1, B1 + B2, B1 + B5, B5, B6, B2 + B6, np.zeros(G)])
    # A blocks: [b0, b3, b4, b3, b4, b3, b7, b4]
    #   (blocks 4..6 = [b4, b3, b7] so t2 can slice a16 at partition 64)
    A = np.stack([B0, B3, B4, B3, B4, B3, B7, B4])
    return W, A


def _combine_mats():
    # sq blocks: [s0..s6] = squares of [b2, b1, b1+b2, b1+b5, b5, b6, b2+b6]
    # Ca out blocks (paired with A): [S+, Gr, Gi, Gi, Gr, Hr, Kr, Hr]
    #   S+ = s1+s0;  Gr = s1-s0;  Gi = s2-s1-s0
    #   Hr = (s3-s1-s4 + s6-s0-s5)/2;  Kr = (s3-s1-s4 - s6+s0+s5)/2
    Wc_a = np.zeros((8, 7))
    Wc_a[0, 0], Wc_a[0, 1] = 1, 1                      # S+
    Wc_a[1, 0], Wc_a[1, 1] = -1, 1                     # Gr
    Wc_a[2, 0], Wc_a[2, 1], Wc_a[2, 2] = -1, -1, 1     # Gi
    Wc_a[3, 0], Wc_a[3, 1], Wc_a[3, 2] = -1, -1, 1     # Gi
    Wc_a[4, 0], Wc_a[4, 1] = -1, 1                     # Gr
    Wc_a[5] = [-0.5, -0.5, 0, 0.5, -0.5, -0.5, 0.5]    # Hr
    Wc_a[6] = [0.5, -0.5, 0, 0.5, -0.5, 0.5, -0.5]     # Kr
    Wc_a[7] = [-0.5, -0.5, 0, 0.5, -0.5, -0.5, 0.5]    # Hr
    # hk blocks: [b2b5, b1b6]
    # Cb out blocks (paired with a16[64:112]=[b4,b3,b7]): [Hi, Hi, Ki]
    #   Hi = hk0-hk1;  Ki = hk0+hk1
    Wc_b = np.zeros((3, 2))
    Wc_b[0] = [1, -1]
    Wc_b[1] = [1, -1]
    Wc_b[2] = [1, 1]
    # t1 products: [b0S+, b3Gr, b4Gi, b3Gi, b4Gr, b3Hr, b7Kr, b4Hr]
    # t2 products: [b4Hi, b3Hi, b7Ki]
    # features: [beta0r, beta1r, beta1i, beta2r, beta2i, beta3r, beta3i]
    Wr_1 = np.zeros((7, 8))
    Wr_1[0, 0] = 1                            # beta0r = b0S+
    Wr_1[1, 1] = Wr_1[1, 2] = 1               # beta1r = b3Gr + b4Gi
    Wr_1[2, 3], Wr_1[2, 4] = 1, -1            # beta1i = b3Gi - b4Gr
    Wr_1[3, 5] = 1                            # beta2r = b3Hr - b4Hi
    Wr_1[4, 7] = 1                            # beta2i = b4Hr + b3Hi
    Wr_1[5, 6] = 1                            # beta3r = b7Kr
    Wr_2 = np.zeros((7, 3))
    Wr_2[3, 0] = -1                           # beta2r -= b4Hi
    Wr_2[4, 1] = 1                            # beta2i += b3Hi
    Wr_2[6, 2] = 1                            # beta3i = b7Ki
    return Wc_a, Wc_b, Wr_1, Wr_2


def _block_diag_lhsT(n_in_blocks, blk=16, coef=None):
    """lhsT[k_partition, m] for a block-structured map."""
    n_out = len(coef)
    lhsT = np.zeros((n_in_blocks * blk, n_out * blk), dtype=np.float32)
    for mb in range(n_out):
        for kb in range(n_in_blocks):
            if coef[mb][kb] != 0.0:
                for c in range(blk):
                    lhsT[kb * blk + c, mb * blk + c] = coef[mb][kb]
    return lhsT


def _build_consts():
    W, A = _form_rows()
    Wc_a, Wc_b, Wr_1, Wr_2 = _combine_mats()

    # form matmuls: input partitions = (16c x 8g), c-major.
    def form_lhsT(rows):
        n_out = rows.shape[0]
        lhsT = np.zeros((128, n_out * 16), dtype=np.float32)
        for j in range(n_out):
            for c in range(16):
                for g in range(G):
                    lhsT[c * G + g, j * 16 + c] = rows[j, g]
        return lhsT

    cW = form_lhsT(W)                                        # [128, 128]
    cA = form_lhsT(A)                                        # [128, 128]
    cCa = _block_diag_lhsT(7, coef=Wc_a)                     # [112, 128]
    cCb = _block_diag_lhsT(2, coef=Wc_b)                     # [32, 48]
    cR1 = _block_diag_lhsT(8, coef=Wr_1)                     # [128, 112]
    cR2 = _block_diag_lhsT(3, coef=Wr_2)                     # [48, 112]
    return cW, cA, cCa, cCb, cR1, cR2


def _fold_weights(conv_w, conv_b):
    w = conv_w.reshape(64, C, 16)
    W7 = np.zeros((64, C, 7), dtype=np.float64)
    W7[..., 0] = w[..., 0] + w[..., 7]
    W7[..., 1] = w[..., 1] + w[..., 6]
    W7[..., 2] = w[..., 9] + w[..., 14]
    W7[..., 3] = w[..., 2] + w[..., 5]
    W7[..., 4] = w[..., 10] + w[..., 13]
    W7[..., 5] = w[..., 3] + w[..., 4]
    W7[..., 6] = w[..., 11] + w[..., 12]
    # conv lhsT per q: [112 = (7f x 16c), 64], packed side by side
    wf = np.zeros((112, NQ * 64), dtype=np.float32)
    for q in range(NQ):
        for f in range(7):
            for cl in range(16):
                wf[f * 16 + cl, q * 64:(q + 1) * 64] = W7[:, q * 16 + cl, f]
    return wf


_PROG_CACHE = {}


def _build_program(loop_n=1):
    import concourse.bass as bass
    import concourse.bacc as bacc
    import concourse.tile as tile
    import concourse.mybir as mybir

    f32 = mybir.dt.float32
    f32r = mybir.dt.float32r
    f16 = mybir.dt.float16
    nc = bacc.Bacc("TRN2", target_bir_lowering=False, debug=False,
                   num_devices=NCORES)

    x_d = nc.dram_tensor("x", [BPC, C * G, HWP], f32r, kind="ExternalInput").ap()
    cW_d = nc.dram_tensor("cW", [128, 128], f32r, kind="ExternalInput").ap()
    cA_d = nc.dram_tensor("cA", [128, 128], f32r, kind="ExternalInput").ap()
    cCa_d = nc.dram_tensor("cCa", [112, 128], f16, kind="ExternalInput").ap()
    cCb_d = nc.dram_tensor("cCb", [32, 48], f16, kind="ExternalInput").ap()
    cR1_d = nc.dram_tensor("cR1", [128, 112], f16, kind="ExternalInput").ap()
    cR2_d = nc.dram_tensor("cR2", [48, 112], f16, kind="ExternalInput").ap()
    wf_d = nc.dram_tensor("wf", [112, NQ * 64], f16, kind="ExternalInput").ap()
    bias_d = nc.dram_tensor("bias", [64, 1], f32, kind="ExternalInput").ap()
    y_d = nc.dram_tensor("y", [BPC, 64, HWP], f32, kind="ExternalOutput").ap()

    LN = mybir.ActivationFunctionType.Ln
    RELU = mybir.ActivationFunctionType.Relu
    IDENT = mybir.ActivationFunctionType.Identity
    MAX = mybir.AluOpType.max
    ADD = mybir.AluOpType.add

    with tile.TileContext(nc) as tc:
        with (
            tc.tile_pool(name="consts", bufs=1) as cpool,
            tc.tile_pool(name="xin", bufs=4) as xpool,
            tc.tile_pool(name="sb", bufs=5) as sbpool,
            tc.tile_pool(name="rl", bufs=4) as rlpool,
            tc.tile_pool(name="yt", bufs=3) as ypool,
            tc.tile_pool(name="psWA", bufs=2, space="PSUM") as psWA,
            tc.tile_pool(name="psC", bufs=1, space="PSUM") as psC,
            tc.tile_pool(name="psR", bufs=1, space="PSUM") as psR,
            tc.tile_pool(name="psY", bufs=1, space="PSUM") as psY,
        ):
            # --- load constants once ---
            cW = cpool.tile([128, 128], f32r, tag="cW")
            cA = cpool.tile([128, 128], f32r, tag="cA")
            cCa = cpool.tile([112, 128], f16, tag="cCa")
            cCb = cpool.tile([32, 48], f16, tag="cCb")
            cR1 = cpool.tile([128, 112], f16, tag="cR1")
            cR2 = cpool.tile([48, 112], f16, tag="cR2")
            wf = cpool.tile([112, NQ * 64], f16, tag="wf")
            bias = cpool.tile([64, 1], f32, tag="bias")
            for t, d in [(cW, cW_d), (cA, cA_d), (cCa, cCa_d), (cCb, cCb_d),
                         (cR1, cR1_d), (cR2, cR2_d), (wf, wf_d),
                         (bias, bias_d)]:
                nc.sync.dma_start(out=t[:], in_=d)

            # Dummy Ln before the loop: makes act-table set 5 (natural_log,
            # which contains Ln+Copy+Identity) resident on the loop-entry
            # path, so the fixpoint hoists BOTH per-iteration
            # InstLoadActFuncSet (2.57us/iter on ACT) out of the loop.
            # The result is stored to y_d[0,0,0:1] (overwritten by the loop)
            # purely so the op isn't dead-code-eliminated.
            warm = cpool.tile([1, 1], f32, tag="warmup")
            nc.scalar.activation(warm[:], bias[0:1, 0:1], LN, bias=1.0)
            nc.sync.dma_start(out=y_d[0, 0:1, 0:1], in_=warm[:])

            import contextlib
            loop_cm = (tc.For_i(0, loop_n, 1) if loop_n > 1
                       else contextlib.nullcontext())
            with loop_cm:
              for b in range(BPC):
                  for jj in range(0, NCHUNK, 2):
                    w = 2 if jj + 1 < NCHUNK else 1
                    sj = jj * S
                    # batched x loads: one [128, w*448] DMA per channel
                    # block covers w spatial chunks (fewer, larger DMAs)
                    xts = []
                    for q in range(NQ):
                        xt = xpool.tile([128, w, S], f32r, tag=f"x{q}")
                        nc.sync.dma_start(
                            out=xt[:], in_=x_d[b, 128 * q:128 * (q + 1),
                                               sj:sj + w * S])
                        xts.append(xt)
                    yt2 = ypool.tile([64, w, S], f32, tag="yt")
                    for dj in range(w):
                      j = jj + dj
                      s0 = j * S
                      pY = None
                      for q in range(NQ):
                          xq = xts[q][:, dj]
                          # form matmuls into one 2-bank PSUM tile
                          pWA = psWA.tile([128, 2, 512], f32, tag="wa")
                          nc.tensor.matmul(pWA[:, 0, 0:S], cW[:], xq)
                          nc.tensor.matmul(pWA[:, 1, 0:S], cA[:], xq)
                          # evacuation W+A -> fp16, split into two ops so
                          # sq/hkb (which need only the W bank) unblock
                          # after 516ns instead of 890ns
                          wa16 = sbpool.tile([128, 2, S], f16, tag="wa16")
                          nc.scalar.copy(wa16[:, 0], pWA[:, 0, 0:S])
                          nc.scalar.copy(wa16[:, 1], pWA[:, 1, 0:S])
                          # squares and hk product on DVE (all-fp16-SBUF).
                          # hk reads its two operand blocks at different
                          # partition bases directly (engine reads follow
                          # the src mem-pattern) -- avoids the 2us-latency
                          # SBUF-to-SBUF partner-alignment DMA that stalled
                          # the per-block dependency chain.
                          sq16 = sbpool.tile([112, S], f16, tag="sq")
                          nc.vector.tensor_mul(sq16[:], wa16[0:112, 0],
                                               wa16[0:112, 0])
                          hkb = sbpool.tile([32, S], f16, tag="hkb")
                          nc.sync.dma_start(out=hkb[:], in_=wa16[64:96, 0])
                          hk16 = sbpool.tile([32, S], f16, tag="hk")
                          nc.vector.tensor_mul(hk16[:], wa16[0:32, 0],
                                               hkb[:])
                          # quadratic combines (fp16 matmuls)
                          pCa = psC.tile([128, S], f32, tag="ca")
                          pCb = psC.tile([48, S], f32, tag="cb")
                          nc.tensor.matmul(pCa[:], cCa[:], sq16[:])
                          nc.tensor.matmul(pCb[:], cCb[:], hk16[:])
                          # round-2 products (one PSUM operand each)
                          t1 = sbpool.tile([128, S], f16, tag="t1")
                          t2 = sbpool.tile([48, S], f16, tag="t2")
                          nc.vector.tensor_mul(t1[:], pCa[:], wa16[:, 1])
                          nc.vector.tensor_mul(t2[:], pCb[:],
                                               wa16[64:112, 1])
                          # beta combine
                          pR = psR.tile([112, S], f32, tag="rpre")
                          nc.tensor.matmul(pR[:], cR1[:], t1[:],
                                           start=True, stop=False)
                          nc.tensor.matmul(pR[:], cR2[:], t2[:],
                                           start=False, stop=True)
                          # ln(1+relu(x)) = max(0, ln(1+x)): MAX on this HW
                          # suppresses NaN (and -inf < 0), so Ln straight
                          # from PSUM (one ACT pass) + a cheap 4x-mode fp16
                          # max on DVE replaces the relu+ln two-pass chain.
                          ln16 = sbpool.tile([112, S], f16, tag="ln16")
                          nc.scalar.activation(ln16[:], pR[:], LN, bias=1.0)
                          rl = rlpool.tile([112, S], f16, tag="rl")
                          nc.vector.tensor_scalar_max(rl[:], ln16[:], 0.0)
                          # conv accumulate
                          if q == 0:
                              pY = psY.tile([64, S], f32, tag="y")
                          nc.tensor.matmul(pY[:],
                                           wf[:, q * 64:(q + 1) * 64],
                                           rl[:], start=(q == 0),
                                           stop=(q == NQ - 1))
                      # evacuate y with bias add on ACT (DVE is the
                      # bottleneck engine); store batched over w chunks
                      nc.scalar.activation(yt2[:, dj], pY[:], IDENT,
                                           bias=bias[:, 0:1])
                    nc.sync.dma_start(out=y_d[b, :, sj:sj + w * S],
                                      in_=yt2[:])
    nc.compile()
    return nc


def kernel(x, conv_w, conv_b):
    from concourse.bass_utils import run_bass_kernel_spmd

    x = np.asarray(x)
    conv_w = np.asarray(conv_w)
    conv_b = np.asarray(conv_b)
    B = x.shape[0]
    xr = np.ascontiguousarray(
        x.reshape(B, C * G, HWP).astype(np.float32))
    cWm, cA, cCa, cCb, cR1, cR2 = _build_consts()
    wf = _fold_weights(conv_w.astype(np.float64), conv_b.astype(np.float64))

    key = "prog"
    if key not in _PROG_CACHE:
        _PROG_CACHE[key] = _build_program()
    nc = _PROG_CACHE[key]

    f16 = np.float16
    consts = dict(cW=cWm, cA=cA,
                  cCa=np.ascontiguousarray(cCa.astype(f16)),
                  cCb=np.ascontiguousarray(cCb.astype(f16)),
                  cR1=np.ascontiguousarray(cR1.astype(f16)),
                  cR2=np.ascontiguousarray(cR2.astype(f16)),
                  wf=np.ascontiguousarray(wf.astype(f16)),
                  bias=np.ascontiguousarray(
                      conv_b.astype(np.float32).reshape(64, 1)))
    in_maps = []
    for i in range(NCORES):
        m = dict(consts)
        m["x"] = np.ascontiguousarray(xr[i * BPC:(i + 1) * BPC])
        in_maps.append(m)

    res = run_bass_kernel_spmd(nc, in_maps, core_ids=list(range(NCORES)))
    y = np.concatenate([res.results[i]["y"] for i in range(NCORES)], axis=0)
    return np.ascontiguousarray(y.reshape(B, 64, 56, 56).astype(np.float32))



# revision 35
# speedup vs baseline: 1.2325x; 1.1363x over previous
"""Trainium2 Bass kernel for nn_BispectrumPool.

Math (validated vs reference):
  F = FFT_8 along the group axis. beta[k] = F1*F[k]*conj(F[1+k mod 8]).
  Due to conjugate symmetry of the real-input FFT:
    beta4=beta3, beta5=beta2, beta6=beta1, beta7=beta0 (real), Im(beta0)=0
  -> only 7 distinct nonzero features per channel:
     [beta0r, beta1r, beta1i, beta2r, beta2i, beta3r, beta3i]
  with (b1,b2)=(Re,Im)F1, (b3,b4)=F2, (b5,b6)=F3, b7=F4(real), b0=F0:
     beta0r = b0*(b1^2+b2^2)
     beta1  = F1^2*conj(F2):    Gr=b1^2-b2^2, Gi=2b1b2
     beta2  = F2*H, H=F1*conj(F3): Hr=b1b5+b2b6, Hi=b2b5-b1b6
     beta3  = F4*K, K=F1*F3:       Kr=b1b5-b2b6, Ki=b2b5+b1b6

Square trick: instead of forming U,V layouts and multiplying, form ONE
W layout  [b2, b1, b1+b2, b1+b5, b5, b6, b2+b6, 0]  and square it on DVE.
All quadratics except Hi,Ki are linear combos of those squares; Hi,Ki come
from one extra 32-partition product [b2b5, b1b6]. This kills one form
matmul and one PSUM evacuation per block vs the direct approach.

Per (b, 448-col chunk, 16-channel block q):
  PE : W, A into one 2-bank PSUM tile; Ca (from squares); Cb (from hk);
       R1/R2 combine; conv accumulate (bias via ones-row, fp16 weights)
  ACT: ONE strided copy evacuates W+A -> fp16 SBUF; ln; relu on odd q
  DVE: sq and hk products (fp16 SBUF), t1, t2 (PSUM operand), relu on
       even q.  (sq on GPSIMD measured slower: Pool shares DVE's SBUF
       port and the contention cost more than DVE's extra op.)
  SP : x-in DMA, yt-out DMA

Distribution: pure data parallel, batch 16 -> 2 per core on 8 cores.
"""

import numpy as np

C, G = 64, 8
HWP = 56 * 56            # 3136
S = 448                  # chunk width (3136 = 7*448)
NCHUNK = HWP // S        # 7
NCORES = 8
BPC = 2                  # batches per core
NQ = 4                   # channel blocks of 16


def _form_rows():
    g = np.arange(G)
    B1 = np.cos(2 * np.pi * g / G)
    B2 = -np.sin(2 * np.pi * g / G)
    B3 = np.cos(4 * np.pi * g / G)
    B4 = -np.sin(4 * np.pi * g / G)
    B5 = np.cos(6 * np.pi * g / G)
    B6 = -np.sin(6 * np.pi * g / G)
    B7 = np.cos(np.pi * g)
    B0 = np.ones(G)
    # W blocks: [b2, b1, b1+b2, b1+b5, b5, b6, b2+b6, 0]
    W = np.stack([B2, B1, B1 + B2, B1 + B5, B5, B6, B2 + B6, np.zeros(G)])
    # A blocks: [b0, b3, b4, b3, b4, b3, b7, b4]
    #   (blocks 4..6 = [b4, b3, b7] so t2 can slice a16 at partition 64)
    A = np.stack([B0, B3, B4, B3, B4, B3, B7, B4])
    return W, A


def _combine_mats():
    # sq blocks: [s0..s6] = squares of [b2, b1, b1+b2, b1+b5, b5, b6, b2+b6]
    # Ca out blocks (paired with A): [S+, Gr, Gi, Gi, Gr, Hr, Kr, Hr]
    #   S+ = s1+s0;  Gr = s1-s0;  Gi = s2-s1-s0
    #   Hr = (s3-s1-s4 + s6-s0-s5)/2;  Kr = (s3-s1-s4 - s6+s0+s5)/2
    Wc_a = np.zeros((8, 7))
    Wc_a[0, 0], Wc_a[0, 1] = 1, 1                      # S+
    Wc_a[1, 0], Wc_a[1, 1] = -1, 1                     # Gr
    Wc_a[2, 0], Wc_a[2, 1], Wc_a[2, 2] = -1, -1, 1     # Gi
    Wc_a[3, 0], Wc_a[3, 1], Wc_a[3, 2] = -1, -1, 1     # Gi
    Wc_a[4, 0], Wc_a[4, 1] = -1, 1                     # Gr
    Wc_a[5] = [-0.5, -0.5, 0, 0.5, -0.5, -0.5, 0.5]    # Hr
    Wc_a[6] = [0.5, -0.5, 0, 0.5, -0.5, 0.5, -0.5]     # Kr
    Wc_a[7] = [-0.5, -0.5, 0, 0.5, -0.5, -0.5, 0.5]    # Hr
    # hk blocks: [b2b5, b1b6]
    # Cb out blocks (paired with a16[64:112]=[b4,b3,b7]): [Hi, Hi, Ki]
    #   Hi = hk0-hk1;  Ki = hk0+hk1
    Wc_b = np.zeros((3, 2))
    Wc_b[0] = [1, -1]
    Wc_b[1] = [1, -1]
    Wc_b[2] = [1, 1]
    # t1 products: [b0S+, b3Gr, b4Gi, b3Gi, b4Gr, b3Hr, b7Kr, b4Hr]
    # t2 products: [b4Hi, b3Hi, b7Ki]
    # features: [beta0r, beta1r, beta1i, beta2r, beta2i, beta3r, beta3i]
    Wr_1 = np.zeros((7, 8))
    Wr_1[0, 0] = 1                            # beta0r = b0S+
    Wr_1[1, 1] = Wr_1[1, 2] = 1               # beta1r = b3Gr + b4Gi
    Wr_1[2, 3], Wr_1[2, 4] = 1, -1            # beta1i = b3Gi - b4Gr
    Wr_1[3, 5] = 1                            # beta2r = b3Hr - b4Hi
    Wr_1[4, 7] = 1                            # beta2i = b4Hr + b3Hi
    Wr_1[5, 6] = 1                            # beta3r = b7Kr
    Wr_2 = np.zeros((7, 3))
    Wr_2[3, 0] = -1                           # beta2r -= b4Hi
    Wr_2[4, 1] = 1                            # beta2i += b3Hi
    Wr_2[6, 2] = 1                            # beta3i = b7Ki
    return Wc_a, Wc_b, Wr_1, Wr_2


def _block_diag_lhsT(n_in_blocks, blk=16, coef=None):
    """lhsT[k_partition, m] for a block-structured map."""
    n_out = len(coef)
    lhsT = np.zeros((n_in_blocks * blk, n_out * blk), dtype=np.float32)
    for mb in range(n_out):
        for kb in range(n_in_blocks):
            if coef[mb][kb] != 0.0:
                for c in range(blk):
                    lhsT[kb * blk + c, mb * blk + c] = coef[mb][kb]
    return lhsT


def _build_consts():
    W, A = _form_rows()
    Wc_a, Wc_b, Wr_1, Wr_2 = _combine_mats()

    # form matmuls: input partitions = (16c x 8g), c-major.
    def form_lhsT(rows):
        n_out = rows.shape[0]
        lhsT = np.zeros((128, n_out * 16), dtype=np.float32)
        for j in range(n_out):
            for c in range(16):
                for g in range(G):
                    lhsT[c * G + g, j * 16 + c] = rows[j, g]
        return lhsT

    cW = form_lhsT(W)                                        # [128, 128]
    cA = form_lhsT(A)                                        # [128, 128]
    cCa = _block_diag_lhsT(7, coef=Wc_a)                     # [112, 128]
    cCb = _block_diag_lhsT(2, coef=Wc_b)                     # [32, 48]
    cR1 = _block_diag_lhsT(8, coef=Wr_1)                     # [128, 112]
    cR2 = _block_diag_lhsT(3, coef=Wr_2)                     # [48, 112]
    return cW, cA, cCa, cCb, cR1, cR2


def _fold_weights(conv_w, conv_b):
    w = conv_w.reshape(64, C, 16)
    W7 = np.zeros((64, C, 7), dtype=np.float64)
    W7[..., 0] = w[..., 0] + w[..., 7]
    W7[..., 1] = w[..., 1] + w[..., 6]
    W7[..., 2] = w[..., 9] + w[..., 14]
    W7[..., 3] = w[..., 2] + w[..., 5]
    W7[..., 4] = w[..., 10] + w[..., 13]
    W7[..., 5] = w[..., 3] + w[..., 4]
    W7[..., 6] = w[..., 11] + w[..., 12]
    # conv lhsT per q: [112 = (7f x 16c), 64], packed side by side
    wf = np.zeros((112, NQ * 64), dtype=np.float32)
    for q in range(NQ):
        for f in range(7):
            for cl in range(16):
                wf[f * 16 + cl, q * 64:(q + 1) * 64] = W7[:, q * 16 + cl, f]
    return wf


_PROG_CACHE = {}


def _build_program(loop_n=1):
    import concourse.bass as bass
    import concourse.bacc as bacc
    import concourse.tile as tile
    import concourse.mybir as mybir

    f32 = mybir.dt.float32
    f32r = mybir.dt.float32r
    f16 = mybir.dt.float16
    nc = bacc.Bacc("TRN2", target_bir_lowering=False, debug=False,
                   num_devices=NCORES)

    x_d = nc.dram_tensor("x", [BPC, C * G, HWP], f32r, kind="ExternalInput").ap()
    cW_d = nc.dram_tensor("cW", [128, 128], f32r, kind="ExternalInput").ap()
    cA_d = nc.dram_tensor("cA", [128, 128], f32r, kind="ExternalInput").ap()
    cCa_d = nc.dram_tensor("cCa", [112, 128], f16, kind="ExternalInput").ap()
    cCb_d = nc.dram_tensor("cCb", [32, 48], f16, kind="ExternalInput").ap()
    cR1_d = nc.dram_tensor("cR1", [128, 112], f16, kind="ExternalInput").ap()
    cR2_d = nc.dram_tensor("cR2", [48, 112], f16, kind="ExternalInput").ap()
    wf_d = nc.dram_tensor("wf", [112, NQ * 64], f16, kind="ExternalInput").ap()
    bias_d = nc.dram_tensor("bias", [64, 1], f32, kind="ExternalInput").ap()
    y_d = nc.dram_tensor("y", [BPC, 64, HWP], f32, kind="ExternalOutput").ap()

    LN = mybir.ActivationFunctionType.Ln
    RELU = mybir.ActivationFunctionType.Relu
    IDENT = mybir.ActivationFunctionType.Identity
    MAX = mybir.AluOpType.max
    ADD = mybir.AluOpType.add

    with tile.TileContext(nc) as tc:
        with (
            tc.tile_pool(name="consts", bufs=1) as cpool,
            tc.tile_pool(name="xin", bufs=4) as xpool,
            tc.tile_pool(name="sb", bufs=5) as sbpool,
            tc.tile_pool(name="rl", bufs=4) as rlpool,
            tc.tile_pool(name="yt", bufs=3) as ypool,
            tc.tile_pool(name="psWA", bufs=2, space="PSUM") as psWA,
            tc.tile_pool(name="psC", bufs=1, space="PSUM") as psC,
            tc.tile_pool(name="psR", bufs=1, space="PSUM") as psR,
            tc.tile_pool(name="psY", bufs=1, space="PSUM") as psY,
        ):
            # --- load constants once ---
            cW = cpool.tile([128, 128], f32r, tag="cW")
            cA = cpool.tile([128, 128], f32r, tag="cA")
            cCa = cpool.tile([112, 128], f16, tag="cCa")
            cCb = cpool.tile([32, 48], f16, tag="cCb")
            cR1 = cpool.tile([128, 112], f16, tag="cR1")
            cR2 = cpool.tile([48, 112], f16, tag="cR2")
            wf = cpool.tile([112, NQ * 64], f16, tag="wf")
            bias = cpool.tile([64, 1], f32, tag="bias")
            for t, d in [(cW, cW_d), (cA, cA_d), (cCa, cCa_d), (cCb, cCb_d),
                         (cR1, cR1_d), (cR2, cR2_d), (wf, wf_d),
                         (bias, bias_d)]:
                nc.sync.dma_start(out=t[:], in_=d)

            # Dummy Ln before the loop: makes act-table set 5 (natural_log,
            # which contains Ln+Copy+Identity) resident on the loop-entry
            # path, so the fixpoint hoists BOTH per-iteration
            # InstLoadActFuncSet (2.57us/iter on ACT) out of the loop.
            # The result is stored to y_d[0,0,0:1] (overwritten by the loop)
            # purely so the op isn't dead-code-eliminated.
            warm = cpool.tile([1, 1], f32, tag="warmup")
            nc.scalar.activation(warm[:], bias[0:1, 0:1], LN, bias=1.0)
            nc.sync.dma_start(out=y_d[0, 0:1, 0:1], in_=warm[:])

            import contextlib
            loop_cm = (tc.For_i(0, loop_n, 1) if loop_n > 1
                       else contextlib.nullcontext())
            with loop_cm:
              for b in range(BPC):
                  for jj in range(0, NCHUNK, 2):
                    w = 2 if jj + 1 < NCHUNK else 1
                    sj = jj * S
                    # batched x loads: one [128, w*448] DMA per channel
                    # block covers w spatial chunks (fewer, larger DMAs)
                    xts = []
                    for q in range(NQ):
                        xt = xpool.tile([128, w, S], f32r, tag=f"x{q}")
                        nc.sync.dma_start(
                            out=xt[:], in_=x_d[b, 128 * q:128 * (q + 1),
                                               sj:sj + w * S])
                        xts.append(xt)
                    yt2 = ypool.tile([64, w, S], f32, tag="yt")
                    for dj in range(w):
                      j = jj + dj
                      s0 = j * S
                      pY = None
                      for q in range(NQ):
                          xq = xts[q][:, dj]
                          # form matmuls into one 2-bank PSUM tile
                          pWA = psWA.tile([128, 2, 512], f32, tag="wa")
                          nc.tensor.matmul(pWA[:, 0, 0:S], cW[:], xq)
                          nc.tensor.matmul(pWA[:, 1, 0:S], cA[:], xq)
                          # single strided evacuation W+A -> fp16
                          wa16 = sbpool.tile([128, 2, S], f16, tag="wa16")
                          nc.scalar.copy(wa16[:], pWA[:, :, 0:S])
                          # squares on DVE (all-fp16-SBUF fast mode); hk
                          # partner blocks [b5,b6] moved to partition 0 via
                          # SBUF-SBUF DMA (tensor_tensor operands must
                          # share a base partition)
                          sq16 = sbpool.tile([112, S], f16, tag="sq")
                          nc.vector.tensor_mul(sq16[:], wa16[0:112, 0],
                                               wa16[0:112, 0])
                          hkb = sbpool.tile([32, S], f16, tag="hkb")
                          nc.sync.dma_start(out=hkb[:], in_=wa16[64:96, 0])
                          hk16 = sbpool.tile([32, S], f16, tag="hk")
                          nc.vector.tensor_mul(hk16[:], wa16[0:32, 0],
                                               hkb[:])
                          # quadratic combines (fp16 matmuls)
                          pCa = psC.tile([128, S], f32, tag="ca")
                          pCb = psC.tile([48, S], f32, tag="cb")
                          nc.tensor.matmul(pCa[:], cCa[:], sq16[:])
                          nc.tensor.matmul(pCb[:], cCb[:], hk16[:])
                          # round-2 products (one PSUM operand each)
                          t1 = sbpool.tile([128, S], f16, tag="t1")
                          t2 = sbpool.tile([48, S], f16, tag="t2")
                          nc.vector.tensor_mul(t1[:], pCa[:], wa16[:, 1])
                          nc.vector.tensor_mul(t2[:], pCb[:],
                                               wa16[64:112, 1])
                          # beta combine
                          pR = psR.tile([112, S], f32, tag="rpre")
                          nc.tensor.matmul(pR[:], cR1[:], t1[:],
                                           start=True, stop=False)
                          nc.tensor.matmul(pR[:], cR2[:], t2[:],
                                           start=False, stop=True)
                          # ln(1+relu(x)) = max(0, ln(1+x)): MAX on this HW
                          # suppresses NaN (and -inf < 0), so Ln straight
                          # from PSUM (one ACT pass) + a cheap 4x-mode fp16
                          # max on DVE replaces the relu+ln two-pass chain.
                          ln16 = sbpool.tile([112, S], f16, tag="ln16")
                          nc.scalar.activation(ln16[:], pR[:], LN, bias=1.0)
                          rl = rlpool.tile([112, S], f16, tag="rl")
                          nc.vector.tensor_scalar_max(rl[:], ln16[:], 0.0)
                          # conv accumulate
                          if q == 0:
                              pY = psY.tile([64, S], f32, tag="y")
                          nc.tensor.matmul(pY[:],
                                           wf[:, q * 64:(q + 1) * 64],
                                           rl[:], start=(q == 0),
                                           stop=(q == NQ - 1))
                      # evacuate y with bias add on ACT (DVE is the
                      # bottleneck engine); store batched over w chunks
                      nc.scalar.activation(yt2[:, dj], pY[:], IDENT,
                                           bias=bias[:, 0:1])
                    nc.sync.dma_start(out=y_d[b, :, sj:sj + w * S],
                                      in_=yt2[:])
    nc.compile()
    return nc


def kernel(x, conv_w, conv_b):
    from concourse.bass_utils import run_bass_kernel_spmd

    x = np.asarray(x)
    conv_w = np.asarray(conv_w)
    conv_b = np.asarray(conv_b)
    B = x.shape[0]
    xr = np.ascontiguousarray(
        x.reshape(B, C * G, HWP).astype(np.float32))
    cWm, cA, cCa, cCb, cR1, cR2 = _build_consts()
    wf = _fold_weights(conv_w.astype(np.float64), conv_b.astype(np.float64))

    key = "prog"
    if key not in _PROG_CACHE:
        _PROG_CACHE[key] = _build_program()
    nc = _PROG_CACHE[key]

    f16 = np.float16
    consts = dict(cW=cWm, cA=cA,
                  cCa=np.ascontiguousarray(cCa.astype(f16)),
                  cCb=np.ascontiguousarray(cCb.astype(f16)),
                  cR1=np.ascontiguousarray(cR1.astype(f16)),
                  cR2=np.ascontiguousarray(cR2.astype(f16)),
                  wf=np.ascontiguousarray(wf.astype(f16)),
                  bias=np.ascontiguousarray(
                      conv_b.astype(np.float32).reshape(64, 1)))
    in_maps = []
    for i in range(NCORES):
        m = dict(consts)
        m["x"] = np.ascontiguousarray(xr[i * BPC:(i + 1) * BPC])
        in_maps.append(m)

    res = run_bass_kernel_spmd(nc, in_maps, core_ids=list(range(NCORES)))
    y = np.concatenate([res.results[i]["y"] for i in range(NCORES)], axis=0)
    return np.ascontiguousarray(y.reshape(B, 64, 56, 56).astype(np.float32))

